# revision 67
# baseline (speedup 1.0000x reference)
"""Trainium2 Bass kernel for nn_AttrAttentionLayer (GAT-style attention layer).

Reference computation per batch element b (N=2048 nodes, F_in=256, F_out=64):
    Wh = h @ W                                  [N, F_out]
    f1 = Wh @ a1 ; f2 = Wh @ a2                 [N]
    e  = leaky_relu(f1[:,None] + f2[None,:], 0.2) * node_type
    att= softmax(where(adj>0, e, -9e15), axis=0)   (softmax over i, per column j)
    out= relu(att @ (Wh * level[:,None]))       [N, F_out]

Sharding: batch dim B=8 -> one batch element per NeuronCore (pure data
parallel, no collectives).

Host-side prep (inside kernel(), per batch element): inputs are re-encoded
element-for-element and transposed for the device:
  node_type -> bf16(node_type^T)          (~0.4% rounding)
  adj       -> bf16((adj^T - 1) * 500)    (0/1 mask -> additive mask
                                           {0, -500}; exact in bf16)
  h         -> bf16(h^T)                  (lets Wh/f1/f2 run as direct PE
                                           matmuls with no transposes)

Engine cost table (per [128, n] bf16 op, verified against the Tile
scheduler's CoreSim cost model): ACT activation (n+224)/1.2 ns; DVE
tensor_tensor 2x mode (n/2+58)/0.96; DVE tensor_scalar 4x mode
(n/4+58)/0.96 (even with two scalar ops); DVE scalar_tensor_tensor only
1x (n+58)/0.96 -- stt is a trap for wide tiles.

Key algebraic identities: node_type >= 0, so
    leaky_relu(z) * nt = leaky_relu(z * nt)
and the mask can be applied BEFORE the leaky:
    exp(leaky(z*nt + M)) with M in {0,-500}: masked entries become
    leaky(x-500) = 0.2(x-500) ~= -100, and exp(-100) underflows to exactly
    0 in bf16 -- identical result to the reference's where()+softmax.
Also leaky(x) = max(x, min(0.2x, 0)), giving a 2-op DVE leaky:
ts(mult 0.2, min 0) at 4x + tt(max) at 2x.

The j-loop balances the two elementwise engines with a uniform COLUMN
split so every step is identical (in-order engines settle into a
constant skew with no per-step bubbles): columns [0:XA=1280] take the
ACT-leaky path u=Prelu(f1bc + bias f2[j]) (the z-add rides the free
bias port, leaky-before-mask), columns [XA:] take the DVE path
v=ts(f1bc+f2b) at 4x with mask-before-leaky; then full-width t=st*nt
and (in-place) t+=adjM TTs on DVE, the 2-op leaky q=ts(0.2t min 0) /
t=tt(max(t,q)) on the XD columns, and one full-width exp on ACT with
accum_out giving the masked softmax denominator for free.

node_type and adjM are packed on the host into ONE [N, 2N] slab so each
j-step is a single 1MB HWDGE DMA issued from SP. Wh tiles are computed
LAZILY inside the loop (2-step lookahead; PE matmul + ACT evict, since
measured ACT has slack) so the pre-loop engine queues stay short; only
f1bc/f2col and two Wh tiles are produced in the head, and f2col comes
from PE (f2 = hT^T @ (W@a2), associativity) with a single DVE eviction.

HW-measured dead ends kept out of the default config (cfg switches
remain): SWDGE/Pool-issued DMAs cost ~+0.5us apiece over HWDGE-SP
(adj_issuer="pool" regressed 82.5->90.8us), and the SDMA-CCE
accum_op=add fused mask (mask_mode="dma_accum") regressed to 104.8us
on HW despite simulating at 65us. scalar_tensor_tensor runs at 1x --
never use it full-width. Halving DMA bytes changed nothing (not
BW-bound); per-step cost is DVE-paced at ~3.5us vs 3.33 modeled.

Head: f1bc [128,N] comes straight out of PE matmuls with a
column-replicated wa1 as lhsT; f2col now ALSO comes from PE
(f2 = hT^T @ (W@a2), associativity) into a [128,NTI] PSUM tile with a
single DVE eviction -- no per-tile DVE stt chain; Wh tiles evict on DVE
(PSUM fp32 copies) keeping ACT nearly idle so the head is DMA-gated.

Tail: per-bank relu evict of h'^T, split 2 banks on ACT / 2 on DVE, then
direct DMA out as bf16 [F_out, N]; the host transposes/casts back.
"""

import sys

import numpy as np

_REPO = "/opt/trn_rl_repo"
if _REPO not in sys.path:
    sys.path.insert(0, _REPO)

import ml_dtypes  # noqa: E402

import concourse.bass as bass  # noqa: E402
import concourse.tile as tile  # noqa: E402
from concourse import bacc, masks, mybir  # noqa: E402

FP32 = mybir.dt.float32
BF16 = mybir.dt.bfloat16

ALPHA = 0.2
MASK_VAL = -500.0
NP_BF16 = ml_dtypes.bfloat16


class Cfg:
    def __init__(self, N=2048, F_in=256, F_out=64, dve_cols=768,
                 slab_bufs=8, prefetch=7, stage_bufs=5, out_bf16=True,
                 mask_mode="tt", adj_issuer="sp"):
        assert N % 128 == 0 and F_in % 128 == 0
        self.N, self.F_in, self.F_out = N, F_in, F_out
        self.NTI = N // 128            # i/j tiles of 128 rows
        self.NFC = F_in // 128         # f-blocks of contraction dim
        self.OC = min(512, N)          # output-chunk width (psum free dim)
        self.NOC = N // self.OC
        self.dve_cols = min(dve_cols, N)   # leaky columns on the DVE path
        assert self.dve_cols % 4 == 0
        self.slab_bufs = slab_bufs
        self.prefetch = min(prefetch, slab_bufs - 1, self.NTI)
        self.stage_bufs = stage_bufs
        self.out_bf16 = out_bf16
        self.mask_mode = mask_mode     # "tt" | "dma_accum"
        self.adj_issuer = adj_issuer   # "sp" | "pool" (SWDGE)
        self.dma_probe_half = False    # perf probe: load half bytes only
        self.probe_steps = None        # perf probe: run only K j-steps
        self.pack_slab = True          # nt|adjM in one [N,2N] slab DMA
        self.st_ahead = 0              # emit st-stage this many steps early
        self.wh_lazy = True            # Wh tiles in-loop (False: all head)


def attn_kernel(tc: tile.TileContext, out_ap, in_aps, cfg: Cfg):
    """Emit the per-core kernel. in_aps: dict name -> bass.AP.

    Expects in_aps["adj"] = bf16((adj^T - 1) * 500) and
    in_aps["node_type"] = bf16(node_type^T)  (see module docstring).
    """
    from contextlib import ExitStack

    nc = tc.nc
    N, F_in, F_out = cfg.N, cfg.F_in, cfg.F_out
    NTI, NFC = cfg.NTI, cfg.NFC
    XD = cfg.dve_cols
    XA = N - XD

    h_d = in_aps["h"]
    ntT_d = in_aps["node_type"]
    adjM_d = in_aps["adj"]
    ntadj_d = in_aps["ntadj"]
    level_d = in_aps["level"]
    W_d = in_aps["W"]

    with ExitStack() as ctx:
        # ---------- persistent SBUF ----------
        persist = ctx.enter_context(tc.tile_pool(name="persist", bufs=1))
        id128 = persist.tile([128, 128], FP32, tag="id128")
        masks.make_identity(nc, id128[:])

        f1bc = persist.tile([128, N], BF16, tag="f1bc")       # f1 bcast rows
        f2col = persist.tile([128, NTI], FP32, tag="f2col")
        wh_all = persist.tile([128, NTI * F_out], FP32, tag="wh")
        cs = persist.tile([128, NTI], FP32, tag="cs")
        inv_cs = persist.tile([128, NTI], FP32, tag="invcs")
        level_sb = persist.tile([128, NTI], FP32, tag="level")
        out_dt = BF16 if cfg.out_bf16 else FP32
        hpT = persist.tile([F_out, N], out_dt, tag="hpT")     # h'^T
        W_sb = persist.tile([128, NFC, F_out], FP32, tag="W")
        W_b = persist.tile([128, NFC, F_out], BF16, tag="Wb")
        wa_b = persist.tile([128, NFC, 2], BF16, tag="wab")
        hT_sb = persist.tile([128, NFC, N], BF16, tag="hT")   # h^T resident

        # 4 PSUM banks accumulate h'^T across the whole j-loop
        ps_hp = ctx.enter_context(tc.tile_pool(name="pshp", bufs=1,
                                               space="PSUM"))
        hp_ps = [ps_hp.tile([F_out, cfg.OC], FP32, tag=f"hp{q}",
                            name=f"hp_ps{q}")
                 for q in range(cfg.NOC)]

        # preload the ACT Prelu/Exp table set at t=0 on a dummy column so
        # the ~1.3us LoadActFuncSet hides under the initial DMAs
        actwarm = persist.tile([128, 2], BF16, tag="actwarm")
        nc.scalar.activation(actwarm[:, 0:1], id128[:, 0:1],
                             mybir.ActivationFunctionType.Prelu,
                             bias=0.0, scale=1.0, alpha=ALPHA)
        nc.scalar.activation(actwarm[:, 1:2], id128[:, 0:1],
                             mybir.ActivationFunctionType.Exp)

        # nt stream ring (SP/HWDGE). mask_mode "tt": adjM gets its own
        # prefetched ring too (issuer per cfg.adj_issuer); "dma_accum":
        # adjM is accum-DMA'd straight onto the t tiles inside the j-loop.
        slab_pool = ctx.enter_context(tc.tile_pool(name="slab",
                                                   bufs=cfg.slab_bufs))
        slabs = {}
        adj_eng = nc.gpsimd if cfg.adj_issuer == "pool" else nc.sync
        adjs = {}
        if cfg.mask_mode == "tt" and not cfg.pack_slab:
            adj_pool = ctx.enter_context(tc.tile_pool(name="adp",
                                                      bufs=cfg.slab_bufs))

        NL = N // 2 if cfg.dma_probe_half else N

        def issue_stream(tj):
            if cfg.pack_slab:
                s_t = slab_pool.tile([128, 2 * N], BF16, tag="slab")
                # one uniform 1MB DMA per tile; splitting early tiles into
                # nt/adj halves measured NEUTRAL for tile 0 and +32us when
                # applied to tiles 0-2 -- extra mid-ring DMAs wreck the
                # HWDGE FIFO pipelining, so keep the stream maximally regular
                nc.sync.dma_start(out=s_t[:],
                                  in_=ntadj_d[tj * 128:(tj + 1) * 128, :])
                slabs[tj] = s_t[:, :N]
                adjs[tj] = s_t[:, N:]
                return
            s_t = slab_pool.tile([128, N], BF16, tag="slab")
            nc.sync.dma_start(out=s_t[:, :NL],
                              in_=ntT_d[tj * 128:(tj + 1) * 128, :NL])
            slabs[tj] = s_t
            if cfg.mask_mode == "tt":
                a_t = adj_pool.tile([128, N], BF16, tag="adj")
                adj_eng.dma_start(out=a_t[:, :NL],
                                  in_=adjM_d[tj * 128:(tj + 1) * 128, :NL])
                adjs[tj] = a_t

        # tiny weight DMAs first (W_b/wa1rep ready before hT lands), then
        # h^T which gates f1bc and with it the whole j-loop
        for c in range(NFC):
            nc.sync.dma_start(out=W_sb[:, c, :],
                              in_=W_d[c * 128:(c + 1) * 128, :])
        wa_sb = persist.tile([128, NFC, 2], FP32, tag="wa")
        nc.sync.dma_start(out=wa_sb[:],
                          in_=in_aps["wa"].rearrange("(c p) k -> p c k",
                                                     p=128))
        HG = min(1024, N)
        for c0 in range(0, N, HG):
            for c in range(NFC):
                nc.sync.dma_start(
                    out=hT_sb[:, c, c0:c0 + HG],
                    in_=h_d[c * 128:(c + 1) * 128, c0:c0 + HG])
        issue_stream(0)
        nc.sync.dma_start(out=level_sb[:, :],
                          in_=level_d.rearrange("(t p) -> p t", p=128))
        for tj in range(1, cfg.prefetch):
            issue_stream(tj)

        # ---------- head: f2col, f1bc; Wh tiles are computed LAZILY in the
        # j-loop (2-tile lookahead, ACT evicts) so the pre-loop ACT/DVE
        # queues stay short ----------
        psW = ctx.enter_context(tc.tile_pool(name="psW", bufs=2,
                                             space="PSUM"))

        def compute_wh(ti, evict_dve=False):
            wh_ps = psW.tile([128, 512], FP32, tag="whps")
            for c in range(NFC):
                nc.tensor.matmul(wh_ps[:, :F_out],
                                 hT_sb[:, c, ti * 128:(ti + 1) * 128],
                                 W_b[:, c, :],
                                 start=(c == 0), stop=(c == NFC - 1))
            if evict_dve:
                nc.vector.tensor_copy(
                    wh_all[:, ti * F_out:(ti + 1) * F_out], wh_ps[:, :F_out])
            else:
                nc.scalar.copy(
                    wh_all[:, ti * F_out:(ti + 1) * F_out], wh_ps[:, :F_out])

        with ExitStack() as p1:
            sb1 = p1.enter_context(tc.tile_pool(name="sb1", bufs=4))
            psF = p1.enter_context(tc.tile_pool(name="psF", bufs=1,
                                                space="PSUM"))

            nc.vector.tensor_copy(W_b[:], W_sb[:])
            nc.vector.tensor_copy(wa_b[:], wa_sb[:])
            ones128 = sb1.tile([128, 128], BF16, tag="ones128")
            nc.vector.memset(ones128[:], 1.0)
            # PE p-state warmup while the hT DMAs land, so the f1bc/Wh
            # matmuls run at full clock; targets the hp PSUM banks, whose
            # first j-loop matmul (start=True) erases the garbage
            for k in range(12):
                w0 = (k * 128) % cfg.OC
                nc.tensor.matmul(
                    hp_ps[k % cfg.NOC][:, w0:w0 + 128],
                    id128[:, :F_out], id128[:], start=True, stop=True,
                    skip_group_check=True)
            # wa1rep[f, m] = wa1[f] for all m: f1bc then comes straight out
            # of PE as wa1rep^T @ hT with no row/broadcast intermediates
            wa1rep = sb1.tile([128, NFC, 128], BF16, tag="warep")
            for c in range(NFC):
                nc.vector.tensor_scalar(
                    out=wa1rep[:, c, :], in0=ones128[:],
                    scalar1=wa_sb[:, c, 0:1],
                    scalar2=None, op0=mybir.AluOpType.mult)

            # f2col[j] = sum_f hT[f,j] * wa2[f]  (PE, j on partitions) --
            # before f1bc so step 0's bias scalar is ready first
            f2_ps = psF.tile([128, NTI], FP32, tag="f2ps")
            for ti in range(NTI):
                for c in range(NFC):
                    nc.tensor.matmul(f2_ps[:, ti:ti + 1],
                                     hT_sb[:, c, ti * 128:(ti + 1) * 128],
                                     wa_b[:, c, 1:2],
                                     start=(c == 0), stop=(c == NFC - 1))
            nc.vector.tensor_copy(f2col[:], f2_ps[:])

            # f1bc[p, i] = f1[i] directly: lhsT = wa1rep (same col repeated)
            # evict engines [ACT, DVE, DVE, ACT]: chunks 0-2 (which gate
            # step 0's Prelu read of [:XA]) land on distinct engines
            for ci, c0 in enumerate(range(0, N, 512)):
                w = min(512, N - c0)
                f_ps = psW.tile([128, 512], FP32, tag="whps")
                for c in range(NFC):
                    nc.tensor.matmul(f_ps[:, :w], wa1rep[:, c, :],
                                     hT_sb[:, c, c0:c0 + w],
                                     start=(c == 0), stop=(c == NFC - 1))
                if ci in (0, 3):
                    nc.scalar.copy(f1bc[:, c0:c0 + w], f_ps[:, :w])
                else:
                    nc.vector.tensor_copy(f1bc[:, c0:c0 + w], f_ps[:, :w])

            # first two Wh tiles in the head; the rest stream in-loop
            # unless wh_lazy is off (keeping hT's last read in the head
            # lets the NEXT For_i iteration's hT DMA prefetch during this
            # iteration's j-loop)
            compute_wh(0, evict_dve=True)
            compute_wh(1, evict_dve=False)
            if not cfg.wh_lazy:
                for ti in range(2, NTI):
                    compute_wh(ti, evict_dve=False)

        st_pre = {}

        # ---------- j-loop (uniform column-split steps) ----------
        with ExitStack() as p3:
            st_pool = p3.enter_context(tc.tile_pool(name="stp",
                                                    bufs=cfg.stage_bufs))
            t_pool = p3.enter_context(tc.tile_pool(name="tp",
                                                   bufs=cfg.stage_bufs))
            q_pool = p3.enter_context(tc.tile_pool(name="qp",
                                                   bufs=cfg.stage_bufs))
            p_pool = p3.enter_context(tc.tile_pool(name="pp",
                                                   bufs=cfg.stage_bufs))
            wl_pool = p3.enter_context(tc.tile_pool(name="wlp", bufs=2))
            TT = nc.vector.tensor_tensor
            TS = nc.vector.tensor_scalar
            Op = mybir.AluOpType

            NTL = cfg.probe_steps or NTI
            for tj in range(NTL):
                if tj + cfg.prefetch < NTL:
                    issue_stream(tj + cfg.prefetch)
                if cfg.wh_lazy and tj + 2 < NTL:
                    compute_wh(tj + 2, evict_dve=False)
                nt_t = slabs.pop(tj)
                f2b = f2col[:, tj:tj + 1]

                # st = leaky(f1bc+f2b) on [0:XA] (ACT Prelu, z-add rides
                # the bias port); st = f1bc+f2b raw on [XA:] (DVE ts 4x).
                # Steps 0-1 are pre-emitted below (lookahead 2) so the st
                # stage for step tj is issued during step tj-2, hiding the
                # engine-queue latency behind the slab DMAs.
                def emit_st(tk):
                    s = st_pool.tile([128, N], BF16, tag="st")
                    fb = f2col[:, tk:tk + 1]
                    if XA > 0:
                        nc.scalar.activation(
                            s[:, :XA], f1bc[:, :XA],
                            mybir.ActivationFunctionType.Prelu,
                            bias=fb, scale=1.0, alpha=ALPHA)
                    if XD > 0:
                        TS(out=s[:, XA:], in0=f1bc[:, XA:], scalar1=fb,
                           scalar2=None, op0=Op.add)
                    return s

                A = cfg.st_ahead
                if A and tj == 0:
                    for tk in range(min(A, NTL)):
                        st_pre[tk] = emit_st(tk)
                if A and tj + A < NTL:
                    st_pre[tj + A] = emit_st(tj + A)
                st_t = st_pre.pop(tj) if A else emit_st(tj)

                # full-width t = st * nt, then the mask add: either a DVE
                # TT from a prefetched adjM ring, or (dma_accum mode) the
                # SDMA CCE adds adjM onto t during its own load
                t_t = t_pool.tile([128, N], BF16, tag="t")
                TT(out=t_t[:], in0=st_t[:], in1=nt_t[:], op=Op.mult)
                if cfg.mask_mode == "tt":
                    adj_t = adjs.pop(tj)
                    TT(out=t_t[:], in0=t_t[:], in1=adj_t[:], op=Op.add)
                else:
                    nc.gpsimd.dma_start(
                        out=t_t[:], in_=adjM_d[tj * 128:(tj + 1) * 128, :],
                        accum_op=Op.add)

                # leaky on the DVE columns (mask-before-leaky there):
                # leaky(x) = max(x, min(0.2x, 0)); in-place on t
                if XD > 0:
                    q_t = q_pool.tile([128, XD], BF16, tag="q")
                    TS(out=q_t[:], in0=t_t[:, XA:], scalar1=ALPHA,
                       scalar2=0.0, op0=Op.mult, op1=Op.min)
                    TT(out=t_t[:, XA:], in0=t_t[:, XA:], in1=q_t[:],
                       op=Op.max)

                # p = exp(w); accum gives the masked softmax denominator
                p_t = p_pool.tile([128, N], BF16, tag="p")
                nc.scalar.activation(p_t[:], t_t[:],
                                     mybir.ActivationFunctionType.Exp,
                                     accum_out=cs[:, tj:tj + 1])

                # approx reciprocal (~18 bits, 5x faster): cs is a positive
                # softmax denominator well inside the safe range
                nc.vector.reciprocal_approx_fast(inv_cs[:, tj:tj + 1],
                                                 cs[:, tj:tj + 1])
                whl2_t = wl_pool.tile([128, F_out], BF16, tag="wl")
                TS(out=whl2_t[:],
                   in0=wh_all[:, tj * F_out:(tj + 1) * F_out],
                   scalar1=level_sb[:, tj:tj + 1],
                   scalar2=inv_cs[:, tj:tj + 1],
                   op0=Op.mult, op1=Op.mult)

                # h'^T[o,i] += whl2[j,o] * p[j,i] into persistent PSUM
                for q in range(cfg.NOC):
                    nc.tensor.matmul(hp_ps[q][:], whl2_t[:],
                                     p_t[:, q * cfg.OC:(q + 1) * cfg.OC],
                                     start=(tj == 0), stop=(tj == NTL - 1),
                                     skip_group_check=True)

        # ---------- tail: per-bank relu evict -> direct DMA out ----
        # the output leaves the device TRANSPOSED ([F_out, N], bf16); the
        # host transposes/casts back -- no PE transposes, no staging copies
        for q in range(cfg.NOC):
            if q % 2 == 0:
                nc.scalar.activation(hpT[:, q * cfg.OC:(q + 1) * cfg.OC],
                                     hp_ps[q][:],
                                     mybir.ActivationFunctionType.Relu)
            else:
                nc.vector.tensor_scalar(
                    out=hpT[:, q * cfg.OC:(q + 1) * cfg.OC],
                    in0=hp_ps[q][:], scalar1=0.0, scalar2=None,
                    op0=mybir.AluOpType.max)
            nc.sync.dma_start(
                out=out_ap[:, q * cfg.OC:(q + 1) * cfg.OC],
                in_=hpT[:, q * cfg.OC:(q + 1) * cfg.OC])


def build(cfg: Cfg, repeats: int = 1):
    """Build the single-core Bass program (same program for all cores).

    repeats > 1 emits the full kernel body that many times in one program
    (used only for timing: per-iteration time = diff of wall times).
    """
    nc = bacc.Bacc("TRN2", target_bir_lowering=False, debug=False)
    N, F_in, F_out = cfg.N, cfg.F_in, cfg.F_out
    in_aps = {
        "h": nc.dram_tensor("h", [F_in, N], BF16, kind="ExternalInput").ap(),
        "node_type": nc.dram_tensor("node_type", [N, N], BF16,
                                    kind="ExternalInput").ap(),
        "adj": nc.dram_tensor("adj", [N, N], BF16, kind="ExternalInput").ap(),
        "ntadj": nc.dram_tensor("ntadj", [N, 2 * N], BF16,
                                kind="ExternalInput").ap(),
        "level": nc.dram_tensor("level", [N], FP32, kind="ExternalInput").ap(),
        "W": nc.dram_tensor("W", [F_in, F_out], FP32, kind="ExternalInput").ap(),
        "a": nc.dram_tensor("a", [2 * F_out, 1], FP32, kind="ExternalInput").ap(),
    }
    in_aps["wa"] = nc.dram_tensor("wa", [F_in, 2], FP32,
                                  kind="ExternalInput").ap()
    out_dt = BF16 if cfg.out_bf16 else FP32
    out_ap = nc.dram_tensor("out", [F_out, N], out_dt,
                            kind="ExternalOutput").ap()
    with tile.TileContext(nc) as tc:
        if repeats == 1:
            attn_kernel(tc, out_ap, in_aps, cfg)
        else:
            with tc.For_i(0, repeats, 1):
                attn_kernel(tc, out_ap, in_aps, cfg)
    nc.compile()
    return nc


_NC_CACHE = {}


def _get_nc(cfg: Cfg, repeats: int = 1):
    key = (cfg.N, cfg.F_in, cfg.F_out, cfg.dve_cols, cfg.slab_bufs,
           cfg.prefetch, cfg.stage_bufs, cfg.out_bf16, cfg.mask_mode,
           cfg.adj_issuer, cfg.probe_steps, cfg.dma_probe_half,
           cfg.pack_slab, cfg.st_ahead, cfg.wh_lazy, repeats)
    if key not in _NC_CACHE:
        _NC_CACHE[key] = build(cfg, repeats)
    return _NC_CACHE[key]


def prep_in_map(inputs: dict, b: int):
    """Host-side shard prep: transpose + re-encode of the N^2 inputs,
    plus the standard constant fold wa = W @ [a1 a2]."""
    adjM = (np.asarray(inputs["adj"][b]).T.astype(np.float32) - 1.0) * 500.0
    W = np.asarray(inputs["W"], dtype=np.float32)
    a = np.asarray(inputs["a"], dtype=np.float32)
    F_out = W.shape[1]
    wa = np.stack([W @ a[:F_out, 0], W @ a[F_out:, 0]], axis=1)
    ntT = np.asarray(inputs["node_type"][b]).T.astype(NP_BF16)
    adjMb = adjM.astype(NP_BF16)
    return {
        "wa": np.ascontiguousarray(wa, dtype=np.float32),
        "h": np.ascontiguousarray(
            np.asarray(inputs["h"][b]).T.astype(NP_BF16)),
        "node_type": np.ascontiguousarray(ntT),
        "adj": np.ascontiguousarray(adjMb),
        "ntadj": np.ascontiguousarray(np.concatenate([ntT, adjMb], axis=1)),
        "level": np.ascontiguousarray(inputs["level"][b], dtype=np.float32),
        "W": np.ascontiguousarray(inputs["W"], dtype=np.float32),
        "a": np.ascontiguousarray(inputs["a"], dtype=np.float32),
    }


def run_on_cores(inputs: dict, cfg: Cfg, trace: bool = False,
                 repeats: int = 1):
    """Shard batch across cores, run, gather. Returns (out[B,N,F_out], bkr)."""
    from concourse.bass_utils import run_bass_kernel_spmd

    B = inputs["h"].shape[0]
    nc = _get_nc(cfg, repeats)
    in_maps = [prep_in_map(inputs, b) for b in range(B)]
    bkr = run_bass_kernel_spmd(nc, in_maps, list(range(B)), trace=trace)
    out = np.stack([np.ascontiguousarray(
        bkr.results[b]["out"].astype(np.float32).T) for b in range(B)],
        axis=0)
    return out, bkr


def kernel(**inputs) -> np.ndarray:
    cfg = Cfg()
    out, _ = run_on_cores(inputs, cfg, trace=False)
    return out.astype(np.float32)


if __name__ == "__main__":
    cfg = Cfg()
    nc = build(cfg)
    print("built ok")
